# revision 1
# baseline (speedup 1.0000x reference)
"""AttentionMemory kernel for Trainium2 (8 NeuronCores, Bass/Tile).

Reference computation (per batch b):
    affinity[n, m] = (2 * mk[:,n]@qk[:,m] - ||mk[:,n]||^2 - ||qk[:,m]||^2) / 8
    out[n, m]      = softmax over n (memory axis)

Softmax over n is invariant to per-column constants, so the -||qk_m||^2
term is dropped.  Logits are produced by an augmented matmul:
    lhsT (stationary) = [0.25 * qk ; -0.125]          -> [65, Mc]
    rhs  (moving)     = [mk        ; a_n   ]          -> [65, N]
    psum[m, n]        = 0.25*dot(qk_m, mk_n) - 0.125*a_n   == logits[m, n]
with a_n = sum_c mk[c,n]^2 precomputed on the host.

Precision: inputs are split hi/lo into bf16 pairs on the host and each
logit tile accumulates three bf16 matmuls in PSUM
    qh@mh + qh@ml + ql@mh      (ql@ml dropped, ~6e-5 logit error)
giving ~1e-4 relative output error at full 1-cycle/row PE throughput
(plain fp32 matmul is 4x slower; float32r is fast but tf32-precision).

Sharding: core c handles batch c//2, query-column half c%2 (communication
free: softmax is over the full n axis which each core holds).  Each core
writes out_c[m, n]; the host transposes to the reference [n, m] layout.

Input DRAM layout is packed by first-use so the head of the pipeline
starts as early as possible:
    q2 [65, 16*252]: per m-strip s, block [qh_s (126) | ql_s (126)]
    m2 [65,  8*1008]: per n-chunk c, block [mh_c (504) | ml_c (504)]

Logits are <= 0, so exp() never overflows and the max-subtraction pass is
skipped (min logit ~ -35 -> exp ~ 1e-16, no underflow in fp32).

Per-core roofline: 32.5 MB f32 output at ~360 GB/s ~= 90 us.  Pipeline:
PE (bf16 matmuls) -> ACT (exp + fused row-sum, PSUM->SBUF) -> DVE
(reciprocal + normalize) -> HWDGE store; the store stream runs gap-free.
"""

import numpy as np

B, CK, H, W = 4, 64, 48, 84
N = H * W            # 4032 memory pixels (softmax axis)
HALF = N // 2        # 2016 query pixels per core
M_STRIP = 126        # output-partition strip size (16 * 126 = 2016)
N_STRIPS = HALF // M_STRIP
K_AUG = CK + 1       # 65: contraction dim incl. the -a_n row

N_QUARTER = N // 4   # 1008: one PSUM tile (2 banks) / one ACT exp call
N_CHUNK = 504        # matmul moving free dim (<=512, one PSUM bank)
N_CHUNKS = N // N_CHUNK  # 8

_CACHE = {}


def _build_nc():
    import concourse.bacc as bacc
    import concourse.mybir as mybir
    import concourse.tile as tile

    f32 = mybir.dt.float32
    bf16 = mybir.dt.bfloat16
    Exp = mybir.ActivationFunctionType.Exp

    nc = bacc.Bacc("TRN2", target_bir_lowering=False, debug=False)

    q2_d = nc.dram_tensor("q2", [K_AUG, 2 * HALF], bf16, kind="ExternalInput")
    m2_d = nc.dram_tensor("m2", [K_AUG, 2 * N], bf16, kind="ExternalInput")
    out_d = nc.dram_tensor("out_c", [HALF, N], f32, kind="ExternalOutput")

    with tile.TileContext(nc) as tc:
        with (
            tc.tile_pool(name="singles", bufs=1) as singles,
            tc.tile_pool(name="psum", bufs=4, space="PSUM") as psum_pool,
            tc.tile_pool(name="exp", bufs=3) as exp_pool,
            tc.tile_pool(name="outs", bufs=4) as out_pool,
            tc.tile_pool(name="stats", bufs=8) as stats_pool,
        ):
            # --- prewarm: ACT exp table load + PE HAM spin-up during the
            # input DMAs -----------------------------------------------------
            wtab = singles.tile([1, 2], f32)
            nc.vector.memset(wtab, 0.0)
            nc.scalar.activation(wtab[:, 1:2], wtab[:, 0:1], Exp)
            wsrc = singles.tile([K_AUG, 256], bf16)
            nc.vector.memset(wsrc, 0.0)
            wps = psum_pool.tile([M_STRIP, 256], f32, tag="ps")
            for _ in range(12):
                nc.tensor.matmul(
                    wps, wsrc[:, :M_STRIP], wsrc, start=True, stop=True
                )

            # --- inputs, staged by first use.  q2 rides the ACT HWDGE ring,
            # m2 the SP ring, so their dispatches overlap ---------------------
            q2_s = singles.tile([K_AUG, 2 * HALF], bf16)
            m2_s = singles.tile([K_AUG, 2 * N], bf16)
            nc.scalar.dma_start(out=q2_s[:, :252], in_=q2_d[:, :252])
            for c0, c1 in ((0, 2), (2, 4), (4, 6), (6, 8)):
                sl = slice(c0 * 1008, c1 * 1008)
                nc.sync.dma_start(out=m2_s[:, sl], in_=m2_d[:, sl])
            nc.sync.dma_start(out=q2_s[:, 252:], in_=q2_d[:, 252:])

            def mh(c):  # rhs hi slice for n-chunk c
                return m2_s[:, c * 1008 : c * 1008 + N_CHUNK]

            def ml(c):  # rhs lo slice for n-chunk c
                return m2_s[:, c * 1008 + N_CHUNK : (c + 1) * 1008]

            for s in range(N_STRIPS):
                m0 = s * M_STRIP
                qh_l = q2_s[:, s * 252 : s * 252 + M_STRIP]
                ql_l = q2_s[:, s * 252 + M_STRIP : (s + 1) * 252]

                exp_t = exp_pool.tile([M_STRIP, N], f32, tag="exp")
                acc = stats_pool.tile([M_STRIP, 8], f32, tag="acc")

                # ACT pieces = pairs of 504-wide n-chunks (one 2-bank PSUM
                # tile / one exp call each)
                pieces = [[0, 1], [2, 3], [4, 5], [6, 7]]
                for pi, piece in enumerate(pieces):
                    k = len(piece)
                    # one PSUM bank (512 cols) per 504-wide chunk; each chunk
                    # starts on a bank boundary — PE writes must not straddle
                    # a bank
                    ps = psum_pool.tile([M_STRIP, 512 * k], f32, tag="ps")
                    for cc, c in enumerate(piece):
                        psl = ps[:, cc * 512 : cc * 512 + N_CHUNK]
                        nc.tensor.matmul(psl, qh_l, mh(c), start=True, stop=False)
                        nc.tensor.matmul(psl, qh_l, ml(c), start=False, stop=False)
                        nc.tensor.matmul(psl, ql_l, mh(c), start=False, stop=True)
                    # exp(logits) PSUM->SBUF with fused per-partition row sum;
                    # the strided 3D views skip the 8 pad columns per bank
                    e0 = piece[0] * N_CHUNK
                    nc.scalar.activation(
                        exp_t[:, e0 : e0 + k * N_CHUNK].rearrange(
                            "p (b c) -> p b c", b=k
                        ),
                        ps.rearrange("p (b c) -> p b c", b=k)[:, :, :N_CHUNK],
                        Exp,
                        accum_out=acc[:, pi : pi + 1],
                    )

                ssum = stats_pool.tile([M_STRIP, 1], f32, tag="ssum")
                nc.vector.reduce_sum(
                    ssum, acc[:, : len(pieces)], axis=mybir.AxisListType.X
                )
                rcp = stats_pool.tile([M_STRIP, 1], f32, tag="rcp")
                nc.vector.reciprocal(rcp, ssum)

                # strip 0 stores in quarters to start the store stream early;
                # steady state stores in 1 MB halves (better real-HW DMA
                # efficiency at equal modeled time)
                out_t = out_pool.tile([M_STRIP, N], f32, tag="out")
                if s == 0:
                    bounds = [0, 1008, 2016, 3024, N]
                else:
                    bounds = [0, N // 2, N]
                for p0, p1 in zip(bounds, bounds[1:]):
                    sl = slice(p0, p1)
                    nc.vector.tensor_scalar_mul(out_t[:, sl], exp_t[:, sl], rcp)
                    nc.sync.dma_start(
                        out=out_d[m0 : m0 + M_STRIP, sl], in_=out_t[:, sl]
                    )

    nc.compile()
    return nc


def _get_nc():
    if "nc" not in _CACHE:
        _CACHE["nc"] = _build_nc()
    return _CACHE["nc"]


def _split_bf16(x: np.ndarray):
    """x (f32) -> (hi, lo) bf16 with hi + lo ~= x (~16 mantissa bits)."""
    import ml_dtypes

    hi = x.astype(ml_dtypes.bfloat16)
    lo = (x - hi.astype(np.float32)).astype(ml_dtypes.bfloat16)
    return hi, lo


def kernel(mk: np.ndarray, qk: np.ndarray) -> np.ndarray:
    import ml_dtypes
    from concourse import bass_utils

    mk = np.asarray(mk, dtype=np.float32).reshape(B, CK, N)
    qk = np.asarray(qk, dtype=np.float32).reshape(B, CK, N)
    a = np.einsum("bcn,bcn->bn", mk, mk)  # sum_c mk^2, [B, N]

    in_maps = []
    for core in range(8):
        b, h = divmod(core, 2)
        mk_aug = np.empty((K_AUG, N), np.float32)
        mk_aug[:CK] = mk[b]
        mk_aug[CK] = a[b]
        mh, ml = _split_bf16(mk_aug)
        # chunk-pair packed: block c = [mh_c | ml_c], each N_CHUNK wide
        m2 = np.empty((K_AUG, 2 * N), ml_dtypes.bfloat16)
        m3 = m2.reshape(K_AUG, N_CHUNKS, 2, N_CHUNK)
        m3[:, :, 0] = mh.reshape(K_AUG, N_CHUNKS, N_CHUNK)
        m3[:, :, 1] = ml.reshape(K_AUG, N_CHUNKS, N_CHUNK)

        qk_aug = np.empty((K_AUG, HALF), np.float32)
        qk_aug[:CK] = 0.25 * qk[b, :, h * HALF : (h + 1) * HALF]
        qk_aug[CK] = -0.125
        qh, ql = _split_bf16(qk_aug)
        ql[CK] = 0  # a_n row must enter exactly once (via qh row 64)
        # strip packed: block s = [qh_s | ql_s], each M_STRIP wide
        q2 = np.empty((K_AUG, 2 * HALF), ml_dtypes.bfloat16)
        q3 = q2.reshape(K_AUG, N_STRIPS, 2, M_STRIP)
        q3[:, :, 0] = qh.reshape(K_AUG, N_STRIPS, M_STRIP)
        q3[:, :, 1] = ql.reshape(K_AUG, N_STRIPS, M_STRIP)

        in_maps.append({"q2": q2, "m2": m2})

    res = bass_utils.run_bass_kernel_spmd(
        _get_nc(), in_maps, core_ids=list(range(8))
    )
    _CACHE["last_results"] = res

    out = np.empty((B, N, N), np.float32)
    for core in range(8):
        b, h = divmod(core, 2)
        out[b, :, h * HALF : (h + 1) * HALF] = res.results[core]["out_c"].T
    return out



# revision 3
# speedup vs baseline: 1.4173x; 1.4173x over previous
"""AttentionMemory kernel for Trainium2 (8 NeuronCores, Bass/Tile).

Reference computation (per batch b):
    affinity[n, m] = (2 * mk[:,n]@qk[:,m] - ||mk[:,n]||^2 - ||qk[:,m]||^2) / 8
    out[n, m]      = softmax over n (memory axis)

Softmax over n is invariant to per-column (m) constants, so the
-||qk_m||^2 term and any global constant are dropped.  Logits come from
one augmented fp32r matmul:
    lhsT (stationary) = [0.25 * qk ; -0.125 ; -0.125]   -> [66, Mc]
    rhs  (moving)     = [mk        ; a_hi   ; a_lo  ]   -> [66, N]
    psum[m, n] = 0.25*dot(qk_m, mk_n) - 0.125*a'_n  == logits[m, n]
with a'_n = sum_c mk[c,n]^2 - mean_n(...) (centering keeps |a'| small so
fp32r rounding of the a-term is negligible; a is additionally split
hi/lo over two rows so the hi part is exact in reduced precision).
fp32r runs at bf16 speed on TRN2 when the moving free dim >= 256
(1 cycle/row), measured logit error ~1.4e-3 -> ~0.14% on the softmax.

The output is stored as bf16 (+0.4% worst-case element error, still
~20x under the 2e-2 gate), halving HBM store traffic vs f32: per-core
16.25 MB at the modeled 360 GB/s aggregate DMA = 45 us.

Engine budget per core (cost model): ACT exp is the bottleneck
(1 elem/cycle/partition @ 1.2 GHz): 64512 cols * 0.83 ns + per-call
overhead ~= 60 us.  Row sums ride a DVE copy pass (tensor_scalar 4x
mode with accum_out, 0.26 ns/col) instead of ACT accum_out reads
(saves 187 ns per ACT call).  PE fp32r ~27-43 us, DVE ~43 us,
DMA ~50 us — all under ACT.

Sharding: core c handles batch c//2, query-column half c%2 (softmax is
over the full n axis which each core holds).  Each core writes
out_c[m, n] bf16; the host casts to f32 and transposes to [n, m].
"""

import numpy as np

B, CK, H, W = 4, 64, 48, 84
N = H * W            # 4032 memory pixels (softmax axis)
HALF = N // 2        # 2016 query pixels per core
M_STRIP = 126        # output-partition strip size (16 * 126 = 2016)
N_STRIPS = HALF // M_STRIP
K_AUG = CK + 2       # 66: contraction dim incl. the a_hi / a_lo rows

N_CHUNK = 504        # matmul moving free dim (one PSUM bank, 8 pad cols)
N_CHUNKS = N // N_CHUNK  # 8
HALF_N = N // 2      # 2016: one ACT exp call / one 4-bank PSUM tile

_CACHE = {}


def _build_nc():
    import concourse.bacc as bacc
    import concourse.mybir as mybir
    import concourse.tile as tile

    f32 = mybir.dt.float32
    f32r = mybir.dt.float32r
    bf16 = mybir.dt.bfloat16
    Exp = mybir.ActivationFunctionType.Exp
    mult = mybir.AluOpType.mult
    add = mybir.AluOpType.add

    nc = bacc.Bacc("TRN2", target_bir_lowering=False, debug=False)

    q_d = nc.dram_tensor("q", [K_AUG, HALF], f32r, kind="ExternalInput")
    m_d = nc.dram_tensor("m", [K_AUG, N], f32r, kind="ExternalInput")
    out_d = nc.dram_tensor("out_c", [HALF, N], bf16, kind="ExternalOutput")

    with tile.TileContext(nc) as tc:
        with (
            tc.tile_pool(name="singles", bufs=1) as singles,
            tc.tile_pool(name="psum", bufs=2, space="PSUM") as psum_pool,
            tc.tile_pool(name="exp", bufs=2) as exp_pool,
            tc.tile_pool(name="outs", bufs=3) as out_pool,
            tc.tile_pool(name="stats", bufs=8) as stats_pool,
        ):
            # --- prewarm: ACT exp table load + PE pstate spin-up during
            # the input DMAs ---------------------------------------------
            wtab = singles.tile([1, 2], f32)
            nc.vector.memset(wtab, 0.0)
            nc.scalar.activation(wtab[:, 1:2], wtab[:, 0:1], Exp)
            wsrc = singles.tile([K_AUG, 256], bf16)
            nc.vector.memset(wsrc, 0.0)
            wps = psum_pool.tile([M_STRIP, 256], f32, tag="ps")
            for _ in range(12):
                nc.tensor.matmul(
                    wps, wsrc[:, :M_STRIP], wsrc, start=True, stop=True
                )

            # --- inputs, staged by first use ----------------------------
            q_s = singles.tile([K_AUG, HALF], f32r)
            m_s = singles.tile([K_AUG, N], f32r)
            nc.scalar.dma_start(out=q_s[:, :M_STRIP], in_=q_d[:, :M_STRIP])
            nc.sync.dma_start(out=m_s[:, :1008], in_=m_d[:, :1008])
            nc.scalar.dma_start(
                out=q_s[:, M_STRIP : 2 * M_STRIP],
                in_=q_d[:, M_STRIP : 2 * M_STRIP],
            )
            for c0 in (1008, 2016, 3024):
                nc.sync.dma_start(
                    out=m_s[:, c0 : c0 + 1008], in_=m_d[:, c0 : c0 + 1008]
                )
            nc.scalar.dma_start(
                out=q_s[:, 2 * M_STRIP :], in_=q_d[:, 2 * M_STRIP :]
            )

            for s in range(N_STRIPS):
                m0 = s * M_STRIP
                q_l = q_s[:, m0 : m0 + M_STRIP]

                exp_t = exp_pool.tile([M_STRIP, N], bf16, tag="exp")
                out_t = out_pool.tile([M_STRIP, N], bf16, tag="out")
                acc = stats_pool.tile([M_STRIP, 2], f32, tag="acc")

                for hh in (0, 1):
                    # 4 chunks -> one 4-bank PSUM tile; each 504-wide chunk
                    # starts on a bank boundary (8 pad cols per bank)
                    ps = psum_pool.tile([M_STRIP, 2048], f32, tag="ps")
                    for cc in range(4):
                        c = 4 * hh + cc
                        nc.tensor.matmul(
                            ps[:, cc * 512 : cc * 512 + N_CHUNK],
                            q_l,
                            m_s[:, c * N_CHUNK : (c + 1) * N_CHUNK],
                            start=True,
                            stop=True,
                        )
                    # exp(logits) PSUM->SBUF bf16; 3D views skip the pad
                    e0 = hh * HALF_N
                    nc.scalar.activation(
                        exp_t[:, e0 : e0 + HALF_N].rearrange(
                            "p (b c) -> p b c", b=4
                        ),
                        ps.rearrange("p (b c) -> p b c", b=4)[:, :, :N_CHUNK],
                        Exp,
                    )
                    # row-sum rides a DVE 4x copy (accum_out); the copy
                    # lands in out_t and is later overwritten by the
                    # normalized values
                    nc.vector.tensor_scalar(
                        out=out_t[:, e0 : e0 + HALF_N],
                        in0=exp_t[:, e0 : e0 + HALF_N],
                        scalar1=1.0,
                        scalar2=None,
                        op0=mult,
                        op1=add,
                        accum_out=acc[:, hh : hh + 1],
                    )

                ssum = stats_pool.tile([M_STRIP, 1], f32, tag="ssum")
                nc.vector.tensor_tensor(
                    out=ssum, in0=acc[:, 0:1], in1=acc[:, 1:2], op=add
                )
                rcp = stats_pool.tile([M_STRIP, 1], f32, tag="rcp")
                nc.vector.reciprocal(rcp, ssum)

                # normalize (DVE 4x) + store per half-strip
                for hh in (0, 1):
                    e0 = hh * HALF_N
                    sl = slice(e0, e0 + HALF_N)
                    nc.vector.tensor_scalar_mul(out_t[:, sl], exp_t[:, sl], rcp)
                    nc.sync.dma_start(
                        out=out_d[m0 : m0 + M_STRIP, sl], in_=out_t[:, sl]
                    )

    nc.compile()
    return nc


def _get_nc():
    if "nc" not in _CACHE:
        _CACHE["nc"] = _build_nc()
    return _CACHE["nc"]


def kernel(mk: np.ndarray, qk: np.ndarray) -> np.ndarray:
    import ml_dtypes
    from concourse import bass_utils

    mk = np.asarray(mk, dtype=np.float32).reshape(B, CK, N)
    qk = np.asarray(qk, dtype=np.float32).reshape(B, CK, N)
    a = np.einsum("bcn,bcn->bn", mk, mk)      # sum_c mk^2, [B, N]
    a -= a.mean(axis=1, keepdims=True)        # softmax-invariant centering
    a_hi = a.astype(ml_dtypes.bfloat16).astype(np.float32)
    a_lo = a - a_hi

    in_maps = []
    for core in range(8):
        b, h = divmod(core, 2)
        m_aug = np.empty((K_AUG, N), np.float32)
        m_aug[:CK] = mk[b]
        m_aug[CK] = a_hi[b]
        m_aug[CK + 1] = a_lo[b]

        q_aug = np.empty((K_AUG, HALF), np.float32)
        q_aug[:CK] = 0.25 * qk[b, :, h * HALF : (h + 1) * HALF]
        q_aug[CK] = -0.125
        q_aug[CK + 1] = -0.125

        in_maps.append({"q": q_aug, "m": m_aug})

    res = bass_utils.run_bass_kernel_spmd(
        _get_nc(), in_maps, core_ids=list(range(8))
    )
    _CACHE["last_results"] = res

    out = np.empty((B, N, N), np.float32)
    for core in range(8):
        b, h = divmod(core, 2)
        out[b, :, h * HALF : (h + 1) * HALF] = (
            res.results[core]["out_c"].astype(np.float32).T
        )
    return out


# revision 4
# speedup vs baseline: 1.4360x; 1.0132x over previous
"""AttentionMemory kernel for Trainium2 (8 NeuronCores, Bass/Tile).

Reference computation (per batch b):
    affinity[n, m] = (2 * mk[:,n]@qk[:,m] - ||mk[:,n]||^2 - ||qk[:,m]||^2) / 8
    out[n, m]      = softmax over n (memory axis)

Softmax over n is invariant to per-column (m) constants, so the
-||qk_m||^2 term and any global constant are dropped.  Logits come from
one augmented fp32r matmul:
    lhsT (stationary) = [0.25 * qk ; -0.125 ; -0.125]   -> [66, Mc]
    rhs  (moving)     = [mk        ; a_hi   ; a_lo  ]   -> [66, N]
    psum[m, n] = 0.25*dot(qk_m, mk_n) - 0.125*a'_n  == logits[m, n]
with a'_n = sum_c mk[c,n]^2 - mean(...) (centering keeps |a'| small so
fp32r rounding of the a-term is negligible; a is additionally split
hi/lo over two rows so the hi part is exact in reduced precision).
fp32r runs at bf16 speed on TRN2 when the moving free dim >= 256
(1 cycle/row); measured logit error ~1.4e-3 -> ~0.14% on the softmax.

The device ships softmax NUMERATORS (exp(logits), bf16) plus per-row
DENOMINATORS (f32 sums); the divide rides the host-side gather pass
that already casts/transposes the result.  bf16 numerator rounding is
a ~0.4% worst-case element error, ~5x under the 2e-2 gate, and halves
HBM store traffic vs f32: per-core 16.25 MB at the modeled 360 GB/s
aggregate DMA = 45 us.

Engine budget per core (cost model): ACT exp is the bottleneck
(1 elem/cycle/partition @ 1.2 GHz): 64512 cols * 0.83 ns + ~185 ns
per-call overhead ~= 60 us busy, gap-free after the first strip.  Row
sums ride DVE tensor_scalar 4x copies (accum_out, 0.26 ns/col) into a
scratch tile, avoiding ACT accum reads (187 ns/call).  PE fp32r ~29 us,
DVE ~22 us, DMA ~50 us — all under ACT.  Output stores depend only on
the exp, so they stream during the strip; the endgame is just the last
strip's sum chain (~3.5 us).

Strip 0 runs in [1,1,2,4]-chunk pieces so the ACT stream starts as
soon as the first 504-column m-chunk lands; the last strip's second
half runs as two 2-chunk pieces to shorten the drain.

Sharding: core c handles batch c//2, query-column half c%2 (softmax is
over the full n axis which each core holds).  Each core writes
out_c[m, n] bf16 + sums[126, 16]; the host divides, casts to f32 and
transposes to the reference [n, m] layout.
"""

import numpy as np

B, CK, H, W = 4, 64, 48, 84
N = H * W            # 4032 memory pixels (softmax axis)
HALF = N // 2        # 2016 query pixels per core
M_STRIP = 126        # output-partition strip size (16 * 126 = 2016)
N_STRIPS = HALF // M_STRIP
K_AUG = CK + 2       # 66: contraction dim incl. the a_hi / a_lo rows

N_CHUNK = 504        # matmul moving free dim (one PSUM bank, 8 pad cols)
N_CHUNKS = N // N_CHUNK  # 8

_CACHE = {}


def _build_nc():
    import concourse.bacc as bacc
    import concourse.mybir as mybir
    import concourse.tile as tile

    f32 = mybir.dt.float32
    f32r = mybir.dt.float32r
    bf16 = mybir.dt.bfloat16
    Exp = mybir.ActivationFunctionType.Exp
    mult = mybir.AluOpType.mult
    add = mybir.AluOpType.add

    nc = bacc.Bacc("TRN2", target_bir_lowering=False, debug=False)

    q_d = nc.dram_tensor("q", [K_AUG, HALF], f32r, kind="ExternalInput")
    m_d = nc.dram_tensor("m", [K_AUG, N], f32r, kind="ExternalInput")
    out_d = nc.dram_tensor("out_c", [HALF, N], bf16, kind="ExternalOutput")
    sums_d = nc.dram_tensor(
        "sums", [M_STRIP, N_STRIPS], f32, kind="ExternalOutput"
    )

    with tile.TileContext(nc) as tc:
        with (
            tc.tile_pool(name="singles", bufs=1) as singles,
            tc.tile_pool(name="psum", bufs=2, space="PSUM") as psum_pool,
            tc.tile_pool(name="exp", bufs=2) as exp_pool,
            tc.tile_pool(name="scratch", bufs=2) as scr_pool,
            tc.tile_pool(name="stats", bufs=4) as stats_pool,
        ):
            # --- prewarm: ACT exp table load + PE pstate spin-up during
            # the input DMAs ---------------------------------------------
            wtab = singles.tile([1, 2], f32)
            nc.vector.memset(wtab, 0.0)
            nc.scalar.activation(wtab[:, 1:2], wtab[:, 0:1], Exp)
            wsrc = singles.tile([K_AUG, 256], bf16)
            nc.vector.memset(wsrc, 0.0)
            wps = psum_pool.tile([M_STRIP, 256], f32, tag="ps")
            for _ in range(12):
                nc.tensor.matmul(
                    wps, wsrc[:, :M_STRIP], wsrc, start=True, stop=True
                )

            # --- inputs, staged by first use; q on the ACT ring so its
            # HWDGE preps overlap the SP ring's m preps -------------------
            q_s = singles.tile([K_AUG, HALF], f32r)
            m_s = singles.tile([K_AUG, N], f32r)
            sums_all = singles.tile([M_STRIP, N_STRIPS], f32)
            nc.scalar.dma_start(out=q_s[:, :M_STRIP], in_=q_d[:, :M_STRIP])
            nc.sync.dma_start(out=m_s[:, :504], in_=m_d[:, :504])
            nc.scalar.dma_start(
                out=q_s[:, M_STRIP : 2 * M_STRIP],
                in_=q_d[:, M_STRIP : 2 * M_STRIP],
            )
            nc.sync.dma_start(out=m_s[:, 504:1008], in_=m_d[:, 504:1008])
            for c0 in (1008, 2016, 3024):
                nc.sync.dma_start(
                    out=m_s[:, c0 : c0 + 1008], in_=m_d[:, c0 : c0 + 1008]
                )
            nc.scalar.dma_start(
                out=q_s[:, 2 * M_STRIP :], in_=q_d[:, 2 * M_STRIP :]
            )

            for s in range(N_STRIPS):
                m0 = s * M_STRIP
                q_l = q_s[:, m0 : m0 + M_STRIP]

                # piece widths in 504-col chunks; sum to 8 per strip
                if s == 0:
                    pieces = [1, 1, 2, 4]
                elif s == N_STRIPS - 1:
                    pieces = [4, 2, 2]
                else:
                    pieces = [4, 4]

                exp_t = exp_pool.tile([M_STRIP, N], bf16, tag="exp")
                acc = stats_pool.tile([M_STRIP, len(pieces)], f32, tag="acc")

                c = 0
                for pi, w in enumerate(pieces):
                    ps = psum_pool.tile([M_STRIP, 512 * w], f32, tag="ps")
                    for cc in range(w):
                        nc.tensor.matmul(
                            ps[:, cc * 512 : cc * 512 + N_CHUNK],
                            q_l,
                            m_s[:, (c + cc) * N_CHUNK : (c + cc + 1) * N_CHUNK],
                            start=True,
                            stop=True,
                        )
                    # exp(logits) PSUM->SBUF bf16; 3D views skip the pad
                    e0 = c * N_CHUNK
                    e1 = (c + w) * N_CHUNK
                    nc.scalar.activation(
                        exp_t[:, e0:e1].rearrange("p (b c) -> p b c", b=w),
                        ps.rearrange("p (b c) -> p b c", b=w)[:, :, :N_CHUNK],
                        Exp,
                    )
                    # store the numerators as soon as the exp lands
                    nc.sync.dma_start(
                        out=out_d[m0 : m0 + M_STRIP, e0:e1], in_=exp_t[:, e0:e1]
                    )
                    # row-sum rides a DVE 4x copy (accum_out) into scratch
                    scr = scr_pool.tile([M_STRIP, 4 * N_CHUNK], bf16, tag="scr")
                    nc.vector.tensor_scalar(
                        out=scr[:, : e1 - e0],
                        in0=exp_t[:, e0:e1],
                        scalar1=1.0,
                        scalar2=None,
                        op0=mult,
                        op1=add,
                        accum_out=acc[:, pi : pi + 1],
                    )
                    c += w

                nc.vector.reduce_sum(
                    sums_all[:, s : s + 1],
                    acc[:, : len(pieces)],
                    axis=mybir.AxisListType.X,
                )

            nc.sync.dma_start(out=sums_d[:, :], in_=sums_all)

    nc.compile()
    return nc


def _get_nc():
    if "nc" not in _CACHE:
        _CACHE["nc"] = _build_nc()
    return _CACHE["nc"]


def kernel(mk: np.ndarray, qk: np.ndarray) -> np.ndarray:
    import ml_dtypes
    from concourse import bass_utils

    mk = np.asarray(mk, dtype=np.float32).reshape(B, CK, N)
    qk = np.asarray(qk, dtype=np.float32).reshape(B, CK, N)
    a = np.einsum("bcn,bcn->bn", mk, mk)      # sum_c mk^2, [B, N]
    a -= a.mean(axis=1, keepdims=True)        # softmax-invariant centering
    a_hi = a.astype(ml_dtypes.bfloat16).astype(np.float32)
    a_lo = a - a_hi

    in_maps = []
    for core in range(8):
        b, h = divmod(core, 2)
        m_aug = np.empty((K_AUG, N), np.float32)
        m_aug[:CK] = mk[b]
        m_aug[CK] = a_hi[b]
        m_aug[CK + 1] = a_lo[b]

        q_aug = np.empty((K_AUG, HALF), np.float32)
        q_aug[:CK] = 0.25 * qk[b, :, h * HALF : (h + 1) * HALF]
        q_aug[CK] = -0.125
        q_aug[CK + 1] = -0.125

        in_maps.append({"q": q_aug, "m": m_aug})

    res = bass_utils.run_bass_kernel_spmd(
        _get_nc(), in_maps, core_ids=list(range(8))
    )
    _CACHE["last_results"] = res

    out = np.empty((B, N, N), np.float32)
    for core in range(8):
        b, h = divmod(core, 2)
        num = res.results[core]["out_c"].astype(np.float32)   # [2016, 4032]
        den = res.results[core]["sums"].astype(np.float32)    # [126, 16]
        num /= den.T.reshape(HALF, 1)                         # strip-major rows
        out[b, :, h * HALF : (h + 1) * HALF] = num.T
    return out


# revision 6
# speedup vs baseline: 1.4669x; 1.0215x over previous
"""AttentionMemory kernel for Trainium2 (8 NeuronCores, Bass/Tile).

Reference computation (per batch b):
    affinity[n, m] = (2 * mk[:,n]@qk[:,m] - ||mk[:,n]||^2 - ||qk[:,m]||^2) / 8
    out[n, m]      = softmax over n (memory axis)

Softmax over n is invariant to per-column (m) constants, so the
-||qk_m||^2 term and any global constant are dropped.  Logits come from
one augmented fp32r matmul:
    lhsT (stationary) = [0.25 * qk ; -0.125 ; -0.125]   -> [66, Mc]
    rhs  (moving)     = [mk        ; a_hi   ; a_lo  ]   -> [66, N]
    psum[m, n] = 0.25*dot(qk_m, mk_n) - 0.125*a'_n  == logits[m, n]
with a'_n = sum_c mk[c,n]^2 - mean(...) (centering keeps |a'| small so
fp32r rounding of the a-term is negligible; a is additionally split
hi/lo over two rows so the hi part is exact in reduced precision).
fp32r runs at bf16 speed on TRN2 when the moving free dim >= 256
(1 cycle/row); measured logit error ~1.4e-3 -> ~0.14% on the softmax.

The device ships softmax NUMERATORS (exp(logits), bf16) plus per-row
DENOMINATORS (f32 sums); the divide rides the host-side gather pass
that already casts/transposes the result.  bf16 numerator rounding is
a ~0.4% worst-case element error, ~5x under the 2e-2 gate, and halves
HBM store traffic vs f32: per-core 16.25 MB at the modeled 360 GB/s
aggregate DMA = 45 us.

Engine budget per core (cost model): ACT exp is the bottleneck
(1 elem/cycle/partition @ 1.2 GHz): 64512 cols * 0.83 ns + ~185 ns
per-call overhead ~= 60 us busy, gap-free after the first strip.  Row
sums ride DVE tensor_scalar 4x copies (accum_out, 0.26 ns/col) into a
scratch tile, avoiding ACT accum reads (187 ns/call).  PE fp32r ~29 us,
DVE ~22 us, DMA ~50 us — all under ACT.  Output stores depend only on
the exp, so they stream during the strip; the endgame is just the last
strip's sum chain (~3.5 us).

Strip 0 runs in [1,1,2,4]-chunk pieces so the ACT stream starts as
soon as the first 504-column m-chunk lands; the last strip's second
half runs as two 2-chunk pieces to shorten the drain.

Sharding: core c handles batch c//2, query-column half c%2 (softmax is
over the full n axis which each core holds).  Each core writes
out_c[m, n] bf16 + sums[126, 16]; the host divides, casts to f32 and
transposes to the reference [n, m] layout.
"""

import numpy as np

B, CK, H, W = 4, 64, 48, 84
N = H * W            # 4032 memory pixels (softmax axis)
HALF = N // 2        # 2016 query pixels per core
M_STRIP = 126        # output-partition strip size (16 * 126 = 2016)
N_STRIPS = HALF // M_STRIP
K_AUG = CK + 2       # 66: contraction dim incl. the a_hi / a_lo rows

N_CHUNK = 504        # matmul moving free dim (one PSUM bank, 8 pad cols)
N_CHUNKS = N // N_CHUNK  # 8

_CACHE = {}


def _build_nc():
    import concourse.bacc as bacc
    import concourse.mybir as mybir
    import concourse.tile as tile

    f32 = mybir.dt.float32
    f32r = mybir.dt.float32r
    bf16 = mybir.dt.bfloat16
    Exp = mybir.ActivationFunctionType.Exp
    mult = mybir.AluOpType.mult
    add = mybir.AluOpType.add

    nc = bacc.Bacc("TRN2", target_bir_lowering=False, debug=False)

    q_d = nc.dram_tensor("q", [K_AUG, HALF], f32r, kind="ExternalInput")
    m_d = nc.dram_tensor("m", [K_AUG, N], f32r, kind="ExternalInput")
    out_d = nc.dram_tensor("out_c", [HALF, N], bf16, kind="ExternalOutput")
    sums_d = nc.dram_tensor(
        "sums", [M_STRIP, N_STRIPS], f32, kind="ExternalOutput"
    )

    with tile.TileContext(nc) as tc:
        with (
            tc.tile_pool(name="singles", bufs=1) as singles,
            tc.tile_pool(name="psum", bufs=2, space="PSUM") as psum_pool,
            tc.tile_pool(name="exp", bufs=3) as exp_pool,
            tc.tile_pool(name="scratch", bufs=2) as scr_pool,
            tc.tile_pool(name="stats", bufs=4) as stats_pool,
        ):
            # --- inputs, staged by first use; q on the ACT ring so its
            # HWDGE preps overlap the SP ring's m preps.  The q loads are
            # issued before the ACT warm so their dispatch isn't queued
            # behind the exp-table load ----------------------------------
            q_s = singles.tile([K_AUG, HALF], f32r)
            m_s = singles.tile([K_AUG, N], f32r)
            sums_all = singles.tile([M_STRIP, N_STRIPS], f32)
            nc.scalar.dma_start(
                out=q_s[:, : 2 * M_STRIP], in_=q_d[:, : 2 * M_STRIP]
            )
            nc.sync.dma_start(out=m_s[:, :504], in_=m_d[:, :504])
            nc.sync.dma_start(out=m_s[:, 504:1008], in_=m_d[:, 504:1008])
            nc.sync.dma_start(out=m_s[:, 1008:2016], in_=m_d[:, 1008:2016])
            nc.sync.dma_start(out=m_s[:, 2016:], in_=m_d[:, 2016:])
            nc.scalar.dma_start(
                out=q_s[:, 2 * M_STRIP :], in_=q_d[:, 2 * M_STRIP :]
            )

            # --- prewarm: ACT exp table load + PE pstate spin-up during
            # the input DMAs ---------------------------------------------
            wtab = singles.tile([1, 2], f32)
            nc.vector.memset(wtab, 0.0)
            nc.scalar.activation(wtab[:, 1:2], wtab[:, 0:1], Exp)
            wsrc = singles.tile([K_AUG, 256], bf16)
            nc.vector.memset(wsrc, 0.0)
            wps = psum_pool.tile([M_STRIP, 256], f32, tag="ps")
            for _ in range(12):
                nc.tensor.matmul(
                    wps, wsrc[:, :M_STRIP], wsrc, start=True, stop=True
                )

            for s in range(N_STRIPS):
                m0 = s * M_STRIP
                q_l = q_s[:, m0 : m0 + M_STRIP]

                # piece widths in 504-col chunks; sum to 8 per strip
                if s == 0:
                    pieces = [1, 1, 2, 4]
                elif s == N_STRIPS - 1:
                    pieces = [4, 2, 2]
                else:
                    pieces = [4, 4]

                exp_t = exp_pool.tile([M_STRIP, N], bf16, tag="exp")
                acc = stats_pool.tile([M_STRIP, len(pieces)], f32, tag="acc")

                c = 0
                for pi, w in enumerate(pieces):
                    ps = psum_pool.tile([M_STRIP, 512 * w], f32, tag="ps")
                    for cc in range(w):
                        nc.tensor.matmul(
                            ps[:, cc * 512 : cc * 512 + N_CHUNK],
                            q_l,
                            m_s[:, (c + cc) * N_CHUNK : (c + cc + 1) * N_CHUNK],
                            start=True,
                            stop=True,
                        )
                    # exp(logits) PSUM->SBUF bf16; 3D views skip the pad
                    e0 = c * N_CHUNK
                    e1 = (c + w) * N_CHUNK
                    nc.scalar.activation(
                        exp_t[:, e0:e1].rearrange("p (b c) -> p b c", b=w),
                        ps.rearrange("p (b c) -> p b c", b=w)[:, :, :N_CHUNK],
                        Exp,
                    )
                    # store the numerators as soon as the exp lands
                    nc.sync.dma_start(
                        out=out_d[m0 : m0 + M_STRIP, e0:e1], in_=exp_t[:, e0:e1]
                    )
                    # row-sum rides a DVE 4x copy (accum_out) into scratch
                    scr = scr_pool.tile([M_STRIP, 4 * N_CHUNK], bf16, tag="scr")
                    nc.vector.tensor_scalar(
                        out=scr[:, : e1 - e0],
                        in0=exp_t[:, e0:e1],
                        scalar1=1.0,
                        scalar2=None,
                        op0=mult,
                        op1=add,
                        accum_out=acc[:, pi : pi + 1],
                    )
                    c += w

                nc.vector.reduce_sum(
                    sums_all[:, s : s + 1],
                    acc[:, : len(pieces)],
                    axis=mybir.AxisListType.X,
                )

            nc.sync.dma_start(out=sums_d[:, :], in_=sums_all)

    nc.compile()
    return nc


def _get_nc():
    if "nc" not in _CACHE:
        _CACHE["nc"] = _build_nc()
    return _CACHE["nc"]


def kernel(mk: np.ndarray, qk: np.ndarray) -> np.ndarray:
    import ml_dtypes
    from concourse import bass_utils

    mk = np.asarray(mk, dtype=np.float32).reshape(B, CK, N)
    qk = np.asarray(qk, dtype=np.float32).reshape(B, CK, N)
    a = np.einsum("bcn,bcn->bn", mk, mk)      # sum_c mk^2, [B, N]
    a -= a.mean(axis=1, keepdims=True)        # softmax-invariant centering
    a_hi = a.astype(ml_dtypes.bfloat16).astype(np.float32)
    a_lo = a - a_hi

    in_maps = []
    for core in range(8):
        b, h = divmod(core, 2)
        m_aug = np.empty((K_AUG, N), np.float32)
        m_aug[:CK] = mk[b]
        m_aug[CK] = a_hi[b]
        m_aug[CK + 1] = a_lo[b]

        q_aug = np.empty((K_AUG, HALF), np.float32)
        q_aug[:CK] = 0.25 * qk[b, :, h * HALF : (h + 1) * HALF]
        q_aug[CK] = -0.125
        q_aug[CK + 1] = -0.125

        in_maps.append({"q": q_aug, "m": m_aug})

    res = bass_utils.run_bass_kernel_spmd(
        _get_nc(), in_maps, core_ids=list(range(8))
    )
    _CACHE["last_results"] = res

    out = np.empty((B, N, N), np.float32)
    for core in range(8):
        b, h = divmod(core, 2)
        num = res.results[core]["out_c"].astype(np.float32)   # [2016, 4032]
        den = res.results[core]["sums"].astype(np.float32)    # [126, 16]
        num /= den.T.reshape(HALF, 1)                         # strip-major rows
        out[b, :, h * HALF : (h + 1) * HALF] = num.T
    return out


# revision 9
# speedup vs baseline: 1.4836x; 1.0114x over previous
"""AttentionMemory kernel for Trainium2 (8 NeuronCores, Bass/Tile).

Reference computation (per batch b):
    affinity[n, m] = (2 * mk[:,n]@qk[:,m] - ||mk[:,n]||^2 - ||qk[:,m]||^2) / 8
    out[n, m]      = softmax over n (memory axis)

Softmax over n is invariant to per-column (m) constants, so the
-||qk_m||^2 term and any global constant are dropped.  Logits come from
one augmented fp32r matmul:
    lhsT (stationary) = [0.25 * qk ; -0.125 ; -0.125]   -> [66, Mc]
    rhs  (moving)     = [mk        ; a_hi   ; a_lo  ]   -> [66, N]
    psum[m, n] = 0.25*dot(qk_m, mk_n) - 0.125*a'_n  == logits[m, n]
with a'_n = sum_c mk[c,n]^2 - mean(...) (centering keeps |a'| small so
fp32r rounding of the a-term is negligible; a is additionally split
hi/lo over two rows so the hi part is exact in reduced precision).
fp32r runs at bf16 speed on TRN2 when the moving free dim >= 256
(1 cycle/row); measured logit error ~1.4e-3 -> ~0.14% on the softmax.

The device ships softmax NUMERATORS (exp(logits), bf16) plus per-row
DENOMINATORS (f32 sums); the divide rides the host-side gather pass
that already casts/transposes the result.  bf16 numerator rounding is
a ~0.4% worst-case element error, ~5x under the 2e-2 gate, and halves
HBM store traffic vs f32: per-core 16.25 MB at the modeled 360 GB/s
aggregate DMA = 45 us.

Engine budget per core (cost model): ACT exp is the bottleneck
(1 elem/cycle/partition @ 1.2 GHz): 64512 cols * 0.83 ns + ~185 ns
per-call overhead ~= 60 us busy, gap-free after the first strip.  Row
sums ride DVE tensor_scalar 4x copies (accum_out, 0.26 ns/col) into a
scratch tile, avoiding ACT accum reads (187 ns/call).  PE fp32r ~29 us,
DVE ~22 us, DMA ~50 us — all under ACT.  Output stores depend only on
the exp, so they stream during the strip; the endgame is just the last
strip's sum chain (~3.5 us).

Strip 0 runs in [1,1,2,4]-chunk pieces so the ACT stream starts as
soon as the first 504-column m-chunk lands; the last strip's second
half runs as two 2-chunk pieces to shorten the drain.

Sharding: core c handles batch c//2, query-column half c%2 (softmax is
over the full n axis which each core holds).  Each core writes
out_c[m, n] bf16 + sums[126, 16]; the host divides, casts to f32 and
transposes to the reference [n, m] layout.
"""

import numpy as np

B, CK, H, W = 4, 64, 48, 84
N = H * W            # 4032 memory pixels (softmax axis)
HALF = N // 2        # 2016 query pixels per core
M_STRIP = 126        # output-partition strip size (16 * 126 = 2016)
N_STRIPS = HALF // M_STRIP
K_AUG = CK + 2       # 66: contraction dim incl. the a_hi / a_lo rows

N_CHUNK = 504        # matmul moving free dim (one PSUM bank, 8 pad cols)
N_CHUNKS = N // N_CHUNK  # 8

_CACHE = {}


def _build_nc():
    import concourse.bacc as bacc
    import concourse.mybir as mybir
    import concourse.tile as tile

    f32 = mybir.dt.float32
    f32r = mybir.dt.float32r
    bf16 = mybir.dt.bfloat16
    Exp = mybir.ActivationFunctionType.Exp
    mult = mybir.AluOpType.mult
    add = mybir.AluOpType.add

    nc = bacc.Bacc("TRN2", target_bir_lowering=False, debug=False)

    q_d = nc.dram_tensor("q", [K_AUG, HALF], f32r, kind="ExternalInput")
    m_d = nc.dram_tensor("m", [K_AUG, N], f32r, kind="ExternalInput")
    out_d = nc.dram_tensor("out_c", [HALF, N], bf16, kind="ExternalOutput")
    sums_d = nc.dram_tensor(
        "sums", [M_STRIP, N_STRIPS], f32, kind="ExternalOutput"
    )

    with tile.TileContext(nc) as tc:
        with (
            tc.tile_pool(name="singles", bufs=1) as singles,
            tc.tile_pool(name="psum", bufs=2, space="PSUM") as psum_pool,
            tc.tile_pool(name="exp", bufs=3) as exp_pool,
            tc.tile_pool(name="scratch", bufs=2) as scr_pool,
            tc.tile_pool(name="stats", bufs=4) as stats_pool,
        ):
            # --- inputs, all on the SP ring in exact first-use order (one
            # shared DMA wire; a big transfer issued on another ring would
            # jump ahead of the m chunks and stall the ACT stream) --------
            q_s = singles.tile([K_AUG, HALF], f32r)
            m_s = singles.tile([K_AUG, N], f32r)
            sums_all = singles.tile([M_STRIP, N_STRIPS], f32)
            nc.sync.dma_start(
                out=q_s[:, : 2 * M_STRIP], in_=q_d[:, : 2 * M_STRIP]
            )
            for c0 in (0, 504, 1008, 1512):
                nc.sync.dma_start(
                    out=m_s[:, c0 : c0 + 504], in_=m_d[:, c0 : c0 + 504]
                )
            for c0 in (2016, 3024):
                nc.sync.dma_start(
                    out=m_s[:, c0 : c0 + 1008], in_=m_d[:, c0 : c0 + 1008]
                )
            nc.sync.dma_start(
                out=q_s[:, 2 * M_STRIP :], in_=q_d[:, 2 * M_STRIP :]
            )

            # --- prewarm: ACT exp table load + PE pstate spin-up during
            # the input DMAs ---------------------------------------------
            wtab = singles.tile([1, 2], f32)
            nc.vector.memset(wtab, 0.0)
            nc.scalar.activation(wtab[:, 1:2], wtab[:, 0:1], Exp)
            wsrc = singles.tile([K_AUG, 256], bf16)
            nc.vector.memset(wsrc, 0.0)
            wps = psum_pool.tile([M_STRIP, 256], f32, tag="ps")
            for _ in range(12):
                nc.tensor.matmul(
                    wps, wsrc[:, :M_STRIP], wsrc, start=True, stop=True
                )

            for s in range(N_STRIPS):
                m0 = s * M_STRIP
                q_l = q_s[:, m0 : m0 + M_STRIP]

                # piece widths in 504-col chunks; sum to 8 per strip
                if s == 0:
                    pieces = [1, 1, 2, 2, 2]
                elif s == N_STRIPS - 1:
                    pieces = [4, 2, 2]
                else:
                    pieces = [4, 4]

                exp_t = exp_pool.tile([M_STRIP, N], bf16, tag="exp")
                acc = stats_pool.tile([M_STRIP, len(pieces)], f32, tag="acc")

                c = 0
                for pi, w in enumerate(pieces):
                    ps = psum_pool.tile([M_STRIP, 512 * w], f32, tag="ps")
                    for cc in range(w):
                        nc.tensor.matmul(
                            ps[:, cc * 512 : cc * 512 + N_CHUNK],
                            q_l,
                            m_s[:, (c + cc) * N_CHUNK : (c + cc + 1) * N_CHUNK],
                            start=True,
                            stop=True,
                        )
                    # exp(logits) PSUM->SBUF bf16; 3D views skip the pad
                    e0 = c * N_CHUNK
                    e1 = (c + w) * N_CHUNK
                    nc.scalar.activation(
                        exp_t[:, e0:e1].rearrange("p (b c) -> p b c", b=w),
                        ps.rearrange("p (b c) -> p b c", b=w)[:, :, :N_CHUNK],
                        Exp,
                    )
                    # store the numerators as soon as the exp lands
                    # (gpsimd SWDGE ring: Pool is otherwise idle, so store
                    # dispatch never contends with ACT/SP/DVE sequencers)
                    nc.gpsimd.dma_start(
                        out=out_d[m0 : m0 + M_STRIP, e0:e1], in_=exp_t[:, e0:e1]
                    )
                    # row-sum rides a DVE 4x copy (accum_out) into scratch
                    scr = scr_pool.tile([M_STRIP, 4 * N_CHUNK], bf16, tag="scr")
                    nc.vector.tensor_scalar(
                        out=scr[:, : e1 - e0],
                        in0=exp_t[:, e0:e1],
                        scalar1=1.0,
                        scalar2=None,
                        op0=mult,
                        op1=add,
                        accum_out=acc[:, pi : pi + 1],
                    )
                    c += w

                nc.vector.reduce_sum(
                    sums_all[:, s : s + 1],
                    acc[:, : len(pieces)],
                    axis=mybir.AxisListType.X,
                )

            nc.gpsimd.dma_start(out=sums_d[:, :], in_=sums_all)

    nc.compile()
    return nc


def _get_nc():
    if "nc" not in _CACHE:
        _CACHE["nc"] = _build_nc()
    return _CACHE["nc"]


def kernel(mk: np.ndarray, qk: np.ndarray) -> np.ndarray:
    import ml_dtypes
    from concourse import bass_utils

    mk = np.asarray(mk, dtype=np.float32).reshape(B, CK, N)
    qk = np.asarray(qk, dtype=np.float32).reshape(B, CK, N)
    a = np.einsum("bcn,bcn->bn", mk, mk)      # sum_c mk^2, [B, N]
    a -= a.mean(axis=1, keepdims=True)        # softmax-invariant centering
    a_hi = a.astype(ml_dtypes.bfloat16).astype(np.float32)
    a_lo = a - a_hi

    in_maps = []
    for core in range(8):
        b, h = divmod(core, 2)
        m_aug = np.empty((K_AUG, N), np.float32)
        m_aug[:CK] = mk[b]
        m_aug[CK] = a_hi[b]
        m_aug[CK + 1] = a_lo[b]

        q_aug = np.empty((K_AUG, HALF), np.float32)
        q_aug[:CK] = 0.25 * qk[b, :, h * HALF : (h + 1) * HALF]
        q_aug[CK] = -0.125
        q_aug[CK + 1] = -0.125

        in_maps.append({"q": q_aug, "m": m_aug})

    res = bass_utils.run_bass_kernel_spmd(
        _get_nc(), in_maps, core_ids=list(range(8))
    )
    _CACHE["last_results"] = res

    out = np.empty((B, N, N), np.float32)
    for core in range(8):
        b, h = divmod(core, 2)
        num = res.results[core]["out_c"].astype(np.float32)   # [2016, 4032]
        den = res.results[core]["sums"].astype(np.float32)    # [126, 16]
        num /= den.T.reshape(HALF, 1)                         # strip-major rows
        out[b, :, h * HALF : (h + 1) * HALF] = num.T
    return out


# revision 10
# speedup vs baseline: 1.5064x; 1.0154x over previous
"""AttentionMemory kernel for Trainium2 (8 NeuronCores, Bass/Tile).

Reference computation (per batch b):
    affinity[n, m] = (2 * mk[:,n]@qk[:,m] - ||mk[:,n]||^2 - ||qk[:,m]||^2) / 8
    out[n, m]      = softmax over n (memory axis)

Softmax over n is invariant to per-column (m) constants, so the
-||qk_m||^2 term and any global constant are dropped.  Logits come from
one augmented fp32r matmul:
    lhsT (stationary) = [0.25 * qk ; -0.125 ; -0.125]   -> [66, Mc]
    rhs  (moving)     = [mk        ; a_hi   ; a_lo  ]   -> [66, N]
    psum[m, n] = 0.25*dot(qk_m, mk_n) - 0.125*a'_n  == logits[m, n]
with a'_n = sum_c mk[c,n]^2 - mean(...) (centering keeps |a'| small so
fp32r rounding of the a-term is negligible; a is additionally split
hi/lo over two rows so the hi part is exact in reduced precision).
fp32r runs at bf16 speed on TRN2 when the moving free dim >= 256
(1 cycle/row); measured logit error ~1.4e-3 -> ~0.14% on the softmax.

The device ships softmax NUMERATORS (exp(logits), bf16) plus per-row
DENOMINATORS (f32 sums); the divide rides the host-side gather pass
that already casts/transposes the result.  bf16 numerator rounding is
a ~0.4% worst-case element error, ~5x under the 2e-2 gate, and halves
HBM store traffic vs f32: per-core 16.25 MB at the modeled 360 GB/s
aggregate DMA = 45 us.

Engine budget per core (cost model): ACT exp is the bottleneck
(1 elem/cycle/partition @ 1.2 GHz): 64512 cols * 0.83 ns + ~185 ns
per-call overhead ~= 60 us busy, gap-free after the first strip.  Row
sums ride DVE tensor_scalar 4x copies (accum_out, 0.26 ns/col) into a
scratch tile, avoiding ACT accum reads (187 ns/call).  PE fp32r ~29 us,
DVE ~22 us, DMA ~50 us — all under ACT.  Output stores depend only on
the exp, so they stream during the strip; the endgame is just the last
strip's sum chain (~3.5 us).

Strip 0 runs in [1,1,2,4]-chunk pieces so the ACT stream starts as
soon as the first 504-column m-chunk lands; the last strip's second
half runs as two 2-chunk pieces to shorten the drain.

Sharding: core c handles batch c//2, query-column half c%2 (softmax is
over the full n axis which each core holds).  Each core writes
out_c[m, n] bf16 + sums[126, 16]; the host divides, casts to f32 and
transposes to the reference [n, m] layout.
"""

import numpy as np

B, CK, H, W = 4, 64, 48, 84
N = H * W            # 4032 memory pixels (softmax axis)
HALF = N // 2        # 2016 query pixels per core
M_STRIP = 126        # output-partition strip size (16 * 126 = 2016)
N_STRIPS = HALF // M_STRIP
K_AUG = CK + 2       # 66: contraction dim incl. the a_hi / a_lo rows

N_CHUNK = 504        # matmul moving free dim (one PSUM bank, 8 pad cols)
N_CHUNKS = N // N_CHUNK  # 8

_CACHE = {}

# Input load schedule: (ring, tensor, col0, width).  Order = program order
# per ring; the single DMA wire serves transfers in ready order.
INPUT_PIECES = [
    ("pool", "m", 0, 504),
    ("sp", "q", 0, 252),
    ("sp", "m", 504, 504),
    ("sp", "m", 1008, 504),
    ("sp", "m", 1512, 504),
    ("sp", "m", 2016, 1008),
    ("sp", "m", 3024, 1008),
    ("sp", "q", 252, 1764),
]
# ACT piece widths (in 504-col chunks) per strip
PIECES_FIRST = [1, 1, 2, 2, 2]
PIECES_LAST = [4, 2, 1, 1]
PIECES_MID = [4, 4]
STORE_RING = "pool"      # steady-state store ring
STORE_RING_LAST = "sp"   # last strip's stores (lowest post-data latency)


def _build_nc():
    import concourse.bacc as bacc
    import concourse.mybir as mybir
    import concourse.tile as tile

    f32 = mybir.dt.float32
    f32r = mybir.dt.float32r
    bf16 = mybir.dt.bfloat16
    Exp = mybir.ActivationFunctionType.Exp
    mult = mybir.AluOpType.mult
    add = mybir.AluOpType.add

    nc = bacc.Bacc("TRN2", target_bir_lowering=False, debug=False)

    q_d = nc.dram_tensor("q", [K_AUG, HALF], f32r, kind="ExternalInput")
    m_d = nc.dram_tensor("m", [K_AUG, N], f32r, kind="ExternalInput")
    out_d = nc.dram_tensor("out_c", [HALF, N], bf16, kind="ExternalOutput")
    sums_d = nc.dram_tensor(
        "sums", [M_STRIP, N_STRIPS], f32, kind="ExternalOutput"
    )

    with tile.TileContext(nc) as tc:
        with (
            tc.tile_pool(name="singles", bufs=1) as singles,
            tc.tile_pool(name="psum", bufs=2, space="PSUM") as psum_pool,
            tc.tile_pool(name="exp", bufs=3) as exp_pool,
            tc.tile_pool(name="scratch", bufs=2) as scr_pool,
            tc.tile_pool(name="stats", bufs=4) as stats_pool,
        ):
            # --- inputs, all on the SP ring in exact first-use order (one
            # shared DMA wire; a big transfer issued on another ring would
            # jump ahead of the m chunks and stall the ACT stream) --------
            q_s = singles.tile([K_AUG, HALF], f32r)
            m_s = singles.tile([K_AUG, N], f32r)
            sums_all = singles.tile([M_STRIP, N_STRIPS], f32)
            rings = {
                "sp": nc.sync,
                "act": nc.scalar,
                "pool": nc.gpsimd,
            }
            for ring, tens, c0, w in INPUT_PIECES:
                src_d, dst_s = (q_d, q_s) if tens == "q" else (m_d, m_s)
                rings[ring].dma_start(
                    out=dst_s[:, c0 : c0 + w], in_=src_d[:, c0 : c0 + w]
                )

            # --- prewarm: ACT exp table load + PE pstate spin-up during
            # the input DMAs ---------------------------------------------
            wtab = singles.tile([1, 2], f32)
            nc.vector.memset(wtab, 0.0)
            nc.scalar.activation(wtab[:, 1:2], wtab[:, 0:1], Exp)
            wsrc = singles.tile([K_AUG, 256], bf16)
            nc.vector.memset(wsrc, 0.0)
            wps = psum_pool.tile([M_STRIP, 256], f32, tag="ps")
            for _ in range(12):
                nc.tensor.matmul(
                    wps, wsrc[:, :M_STRIP], wsrc, start=True, stop=True
                )

            for s in range(N_STRIPS):
                m0 = s * M_STRIP
                q_l = q_s[:, m0 : m0 + M_STRIP]

                # piece widths in 504-col chunks; sum to 8 per strip
                if s == 0:
                    pieces = PIECES_FIRST
                elif s == N_STRIPS - 1:
                    pieces = PIECES_LAST
                else:
                    pieces = PIECES_MID

                exp_t = exp_pool.tile([M_STRIP, N], bf16, tag="exp")
                acc = stats_pool.tile([M_STRIP, len(pieces)], f32, tag="acc")

                c = 0
                for pi, w in enumerate(pieces):
                    ps = psum_pool.tile([M_STRIP, 512 * w], f32, tag="ps")
                    for cc in range(w):
                        nc.tensor.matmul(
                            ps[:, cc * 512 : cc * 512 + N_CHUNK],
                            q_l,
                            m_s[:, (c + cc) * N_CHUNK : (c + cc + 1) * N_CHUNK],
                            start=True,
                            stop=True,
                        )
                    # exp(logits) PSUM->SBUF bf16; 3D views skip the pad
                    e0 = c * N_CHUNK
                    e1 = (c + w) * N_CHUNK
                    nc.scalar.activation(
                        exp_t[:, e0:e1].rearrange("p (b c) -> p b c", b=w),
                        ps.rearrange("p (b c) -> p b c", b=w)[:, :, :N_CHUNK],
                        Exp,
                    )
                    # store the numerators as soon as the exp lands
                    # (Pool SWDGE ring in steady state: Pool is otherwise
                    # idle so store dispatch never contends; the drain-
                    # critical last strip uses the lowest-latency ring)
                    sring = STORE_RING_LAST if s == N_STRIPS - 1 else STORE_RING
                    rings[sring].dma_start(
                        out=out_d[m0 : m0 + M_STRIP, e0:e1], in_=exp_t[:, e0:e1]
                    )
                    # row-sum rides a DVE 4x copy (accum_out) into scratch
                    scr = scr_pool.tile([M_STRIP, 4 * N_CHUNK], bf16, tag="scr")
                    nc.vector.tensor_scalar(
                        out=scr[:, : e1 - e0],
                        in0=exp_t[:, e0:e1],
                        scalar1=1.0,
                        scalar2=None,
                        op0=mult,
                        op1=add,
                        accum_out=acc[:, pi : pi + 1],
                    )
                    c += w

                nc.vector.reduce_sum(
                    sums_all[:, s : s + 1],
                    acc[:, : len(pieces)],
                    axis=mybir.AxisListType.X,
                )

            nc.scalar.dma_start(out=sums_d[:, :], in_=sums_all)

    nc.compile()
    return nc


def _get_nc():
    if "nc" not in _CACHE:
        _CACHE["nc"] = _build_nc()
    return _CACHE["nc"]


def kernel(mk: np.ndarray, qk: np.ndarray) -> np.ndarray:
    import ml_dtypes
    from concourse import bass_utils

    mk = np.asarray(mk, dtype=np.float32).reshape(B, CK, N)
    qk = np.asarray(qk, dtype=np.float32).reshape(B, CK, N)
    a = np.einsum("bcn,bcn->bn", mk, mk)      # sum_c mk^2, [B, N]
    a -= a.mean(axis=1, keepdims=True)        # softmax-invariant centering
    a_hi = a.astype(ml_dtypes.bfloat16).astype(np.float32)
    a_lo = a - a_hi

    in_maps = []
    for core in range(8):
        b, h = divmod(core, 2)
        m_aug = np.empty((K_AUG, N), np.float32)
        m_aug[:CK] = mk[b]
        m_aug[CK] = a_hi[b]
        m_aug[CK + 1] = a_lo[b]

        q_aug = np.empty((K_AUG, HALF), np.float32)
        q_aug[:CK] = 0.25 * qk[b, :, h * HALF : (h + 1) * HALF]
        q_aug[CK] = -0.125
        q_aug[CK + 1] = -0.125

        in_maps.append({"q": q_aug, "m": m_aug})

    res = bass_utils.run_bass_kernel_spmd(
        _get_nc(), in_maps, core_ids=list(range(8))
    )
    _CACHE["last_results"] = res

    out = np.empty((B, N, N), np.float32)
    for core in range(8):
        b, h = divmod(core, 2)
        num = res.results[core]["out_c"].astype(np.float32)   # [2016, 4032]
        den = res.results[core]["sums"].astype(np.float32)    # [126, 16]
        num /= den.T.reshape(HALF, 1)                         # strip-major rows
        out[b, :, h * HALF : (h + 1) * HALF] = num.T
    return out


# revision 12
# speedup vs baseline: 1.5113x; 1.0033x over previous
"""AttentionMemory kernel for Trainium2 (8 NeuronCores, Bass/Tile).

Reference computation (per batch b):
    affinity[n, m] = (2 * mk[:,n]@qk[:,m] - ||mk[:,n]||^2 - ||qk[:,m]||^2) / 8
    out[n, m]      = softmax over n (memory axis)

Softmax over n is invariant to per-column (m) constants, so the
-||qk_m||^2 term and any global constant are dropped.  Logits come from
one augmented fp32r matmul:
    lhsT (stationary) = [0.25 * qk ; -0.125 ; -0.125]   -> [66, Mc]
    rhs  (moving)     = [mk        ; a_hi   ; a_lo  ]   -> [66, N]
    psum[m, n] = 0.25*dot(qk_m, mk_n) - 0.125*a'_n  == logits[m, n]
with a'_n = sum_c mk[c,n]^2 - mean(...) (centering keeps |a'| small so
fp32r rounding of the a-term is negligible; a is additionally split
hi/lo over two rows so the hi part is exact in reduced precision).
fp32r runs at bf16 speed on TRN2 when the moving free dim >= 256
(1 cycle/row); measured logit error ~1.4e-3 -> ~0.14% on the softmax.

The device ships softmax NUMERATORS (exp(logits), bf16) plus per-row
DENOMINATORS (f32 sums); the divide rides the host-side gather pass
that already casts/transposes the result.  bf16 numerator rounding is
a ~0.4% worst-case element error, ~5x under the 2e-2 gate, and halves
HBM store traffic vs f32: per-core 16.25 MB at the modeled 360 GB/s
aggregate DMA = 45 us.

Engine budget per core (cost model): ACT exp is the bottleneck
(1 elem/cycle/partition @ 1.2 GHz): 64512 cols * 0.83 ns + ~185 ns
per-call overhead ~= 60 us busy, gap-free after the first strip.  Row
sums ride DVE tensor_scalar 4x copies (accum_out, 0.26 ns/col) into a
scratch tile, avoiding ACT accum reads (187 ns/call).  PE fp32r ~29 us,
DVE ~22 us, DMA ~50 us — all under ACT.  Output stores depend only on
the exp, so they stream during the strip; the endgame is just the last
strip's sum chain (~3.5 us).

Strip 0 runs in [1,1,2,4]-chunk pieces so the ACT stream starts as
soon as the first 504-column m-chunk lands; the last strip's second
half runs as two 2-chunk pieces to shorten the drain.

Sharding: core c handles batch c//2, query-column half c%2 (softmax is
over the full n axis which each core holds).  Each core writes
out_c[m, n] bf16 + sums[126, 16]; the host divides, casts to f32 and
transposes to the reference [n, m] layout.
"""

import numpy as np

B, CK, H, W = 4, 64, 48, 84
N = H * W            # 4032 memory pixels (softmax axis)
HALF = N // 2        # 2016 query pixels per core
M_STRIP = 126        # output-partition strip size (16 * 126 = 2016)
N_STRIPS = HALF // M_STRIP
K_AUG = CK + 2       # 66: contraction dim incl. the a_hi / a_lo rows

N_CHUNK = 504        # matmul moving free dim (one PSUM bank, 8 pad cols)
N_CHUNKS = N // N_CHUNK  # 8

_CACHE = {}

# Input load schedule: (ring, tensor, col0, width).  Order = program order
# per ring; the single DMA wire serves transfers in ready order.
INPUT_PIECES = [
    ("pool", "q", 0, 252),
    ("sp", "m", 0, 504),
    ("sp", "m", 504, 504),
    ("sp", "m", 1008, 1008),
    ("sp", "m", 2016, 1008),
    ("sp", "m", 3024, 1008),
    ("sp", "q", 252, 1764),
]
# ACT piece widths (in 504-col chunks) per strip
PIECES_FIRST = [1, 1, 2, 2, 2]
PIECES_LAST = [4, 2, 1, 1]
PIECES_MID = [4, 4]
STORE_RING = "pool"      # steady-state store ring
STORE_RING_LAST = "sp"   # last strip's stores (lowest post-data latency)
N_WARM = 12              # PE pstate spin-up matmuls
WARM_MEMSET_RING = "vector"  # engine for the warm-source memset


def _build_nc():
    import concourse.bacc as bacc
    import concourse.mybir as mybir
    import concourse.tile as tile

    f32 = mybir.dt.float32
    f32r = mybir.dt.float32r
    bf16 = mybir.dt.bfloat16
    Exp = mybir.ActivationFunctionType.Exp
    mult = mybir.AluOpType.mult
    add = mybir.AluOpType.add

    nc = bacc.Bacc("TRN2", target_bir_lowering=False, debug=False)

    q_d = nc.dram_tensor("q", [K_AUG, HALF], f32r, kind="ExternalInput")
    m_d = nc.dram_tensor("m", [K_AUG, N], f32r, kind="ExternalInput")
    out_d = nc.dram_tensor("out_c", [HALF, N], bf16, kind="ExternalOutput")
    sums_d = nc.dram_tensor(
        "sums", [M_STRIP, N_STRIPS], f32, kind="ExternalOutput"
    )

    with tile.TileContext(nc) as tc:
        with (
            tc.tile_pool(name="singles", bufs=1) as singles,
            tc.tile_pool(name="psum", bufs=2, space="PSUM") as psum_pool,
            tc.tile_pool(name="exp", bufs=3) as exp_pool,
            tc.tile_pool(name="scratch", bufs=2) as scr_pool,
            tc.tile_pool(name="stats", bufs=4) as stats_pool,
        ):
            # --- inputs, all on the SP ring in exact first-use order (one
            # shared DMA wire; a big transfer issued on another ring would
            # jump ahead of the m chunks and stall the ACT stream) --------
            q_s = singles.tile([K_AUG, HALF], f32r)
            m_s = singles.tile([K_AUG, N], f32r)
            sums_all = singles.tile([M_STRIP, N_STRIPS], f32)
            rings = {
                "sp": nc.sync,
                "act": nc.scalar,
                "pool": nc.gpsimd,
            }
            for ring, tens, c0, w in INPUT_PIECES:
                src_d, dst_s = (q_d, q_s) if tens == "q" else (m_d, m_s)
                rings[ring].dma_start(
                    out=dst_s[:, c0 : c0 + w], in_=src_d[:, c0 : c0 + w]
                )

            # --- prewarm: ACT exp table load + PE pstate spin-up during
            # the input DMAs ---------------------------------------------
            wtab = singles.tile([1, 2], f32)
            nc.vector.memset(wtab, 0.0)
            nc.scalar.activation(wtab[:, 1:2], wtab[:, 0:1], Exp)
            wsrc = singles.tile([K_AUG, 256], bf16)
            getattr(nc, WARM_MEMSET_RING).memset(wsrc, 0.0)
            wps = psum_pool.tile([M_STRIP, 256], f32, tag="ps")
            for _ in range(N_WARM):
                nc.tensor.matmul(
                    wps, wsrc[:, :M_STRIP], wsrc, start=True, stop=True
                )

            for s in range(N_STRIPS):
                m0 = s * M_STRIP
                q_l = q_s[:, m0 : m0 + M_STRIP]

                # piece widths in 504-col chunks; sum to 8 per strip
                if s == 0:
                    pieces = PIECES_FIRST
                elif s == N_STRIPS - 1:
                    pieces = PIECES_LAST
                else:
                    pieces = PIECES_MID

                exp_t = exp_pool.tile([M_STRIP, N], bf16, tag="exp")
                acc = stats_pool.tile([M_STRIP, len(pieces)], f32, tag="acc")

                c = 0
                for pi, w in enumerate(pieces):
                    ps = psum_pool.tile([M_STRIP, 512 * w], f32, tag="ps")
                    for cc in range(w):
                        nc.tensor.matmul(
                            ps[:, cc * 512 : cc * 512 + N_CHUNK],
                            q_l,
                            m_s[:, (c + cc) * N_CHUNK : (c + cc + 1) * N_CHUNK],
                            start=True,
                            stop=True,
                        )
                    # exp(logits) PSUM->SBUF bf16; 3D views skip the pad
                    e0 = c * N_CHUNK
                    e1 = (c + w) * N_CHUNK
                    nc.scalar.activation(
                        exp_t[:, e0:e1].rearrange("p (b c) -> p b c", b=w),
                        ps.rearrange("p (b c) -> p b c", b=w)[:, :, :N_CHUNK],
                        Exp,
                    )
                    # store the numerators as soon as the exp lands
                    # (Pool SWDGE ring in steady state: Pool is otherwise
                    # idle so store dispatch never contends; the drain-
                    # critical last strip uses the lowest-latency ring)
                    sring = STORE_RING_LAST if s == N_STRIPS - 1 else STORE_RING
                    rings[sring].dma_start(
                        out=out_d[m0 : m0 + M_STRIP, e0:e1], in_=exp_t[:, e0:e1]
                    )
                    # row-sum rides a DVE 4x copy (accum_out) into scratch
                    scr = scr_pool.tile([M_STRIP, 4 * N_CHUNK], bf16, tag="scr")
                    nc.vector.tensor_scalar(
                        out=scr[:, : e1 - e0],
                        in0=exp_t[:, e0:e1],
                        scalar1=1.0,
                        scalar2=None,
                        op0=mult,
                        op1=add,
                        accum_out=acc[:, pi : pi + 1],
                    )
                    c += w

                nc.vector.reduce_sum(
                    sums_all[:, s : s + 1],
                    acc[:, : len(pieces)],
                    axis=mybir.AxisListType.X,
                )

            nc.scalar.dma_start(out=sums_d[:, :], in_=sums_all)

    nc.compile()
    return nc


def _get_nc():
    if "nc" not in _CACHE:
        _CACHE["nc"] = _build_nc()
    return _CACHE["nc"]


def kernel(mk: np.ndarray, qk: np.ndarray) -> np.ndarray:
    import ml_dtypes
    from concourse import bass_utils

    mk = np.asarray(mk, dtype=np.float32).reshape(B, CK, N)
    qk = np.asarray(qk, dtype=np.float32).reshape(B, CK, N)
    a = np.einsum("bcn,bcn->bn", mk, mk)      # sum_c mk^2, [B, N]
    a -= a.mean(axis=1, keepdims=True)        # softmax-invariant centering
    a_hi = a.astype(ml_dtypes.bfloat16).astype(np.float32)
    a_lo = a - a_hi

    in_maps = []
    for core in range(8):
        b, h = divmod(core, 2)
        m_aug = np.empty((K_AUG, N), np.float32)
        m_aug[:CK] = mk[b]
        m_aug[CK] = a_hi[b]
        m_aug[CK + 1] = a_lo[b]

        q_aug = np.empty((K_AUG, HALF), np.float32)
        q_aug[:CK] = 0.25 * qk[b, :, h * HALF : (h + 1) * HALF]
        q_aug[CK] = -0.125
        q_aug[CK + 1] = -0.125

        in_maps.append({"q": q_aug, "m": m_aug})

    res = bass_utils.run_bass_kernel_spmd(
        _get_nc(), in_maps, core_ids=list(range(8))
    )
    _CACHE["last_results"] = res

    out = np.empty((B, N, N), np.float32)
    for core in range(8):
        b, h = divmod(core, 2)
        num = res.results[core]["out_c"].astype(np.float32)   # [2016, 4032]
        den = res.results[core]["sums"].astype(np.float32)    # [126, 16]
        num /= den.T.reshape(HALF, 1)                         # strip-major rows
        out[b, :, h * HALF : (h + 1) * HALF] = num.T
    return out
